# revision 1
# baseline (speedup 1.0000x reference)
"""Multi-head causal self-attention (torch nn.MultiheadAttention semantics)
on 8 Trainium2 NeuronCores.

Problem: x [2, 2048, 1024], 16 heads, head dim 64, fp32, causal, p_drop=0.

Sharding: 2 batch groups x 4-way head tensor-parallel.
  core c: batch b = c // 4, heads [lane*4, lane*4+4) with lane = c % 4.
Each core computes q/k/v projections for its 4 heads, flash-style causal
attention (S^T score layout, no-max softmax — scores are O(1) here), and its
partial out-projection. The host sums the 4 partials per batch and adds b_out
(this is the all-reduce of the tensor-parallel decomposition, done on host
since the harness contract is full-input -> full-output).

All matmuls run in f32r (reduced-precision fp32 mode of the PE): same
throughput as bf16 (1 cycle/row at moving free >= 256) with ~1.5e-4 matmul
relative error; end-to-end absmax rel err vs the fp32 reference is ~1e-4.

Per-core program details:
  qkT [2*DQ, S] = (wqkT.T @ xT) + bqk  (q and k kept transposed: [dh, seq])
  v' per sk-block: [128, 4*(64+1)] — per-head v with an appended ones column,
     so the PV matmul's row 64 accumulates the softmax denominator for free.
  scores^T block [sk 128, sq 512] = kT.T @ qT; P = exp(s/8) (f32r);
     diagonal blocks multiplied by a precomputed 0/1 causal mask;
  out^T psum [65, 512] accumulates v'.T @ P over sk blocks; row 64 = denom;
     normalized via reciprocal + gpsimd partition-broadcast + DVE mul.
  out [S, DM] partial = OT.T @ woT per 128-row block.
"""

import os
from contextlib import ExitStack
from dataclasses import dataclass

import numpy as np

import concourse.bass as bass
import concourse.tile as tile
from concourse import bacc, mybir
from concourse.bass_utils import run_bass_kernel_spmd

F32 = mybir.dt.float32
F32R = mybir.dt.float32r
AF = mybir.ActivationFunctionType

B = 2
S = 2048
DM = 1024
N_HEADS = 16
DH = 64
N_CORES = 8
CPG = 4  # cores per group (tensor-parallel width over heads)
HPC = N_HEADS // CPG  # heads per core
DQ = HPC * DH
SPAN = 512
SB = 128
NDM = DM // 128
NSPAN = S // SPAN
NSB = S // SB
SBS = SPAN // SB
NQK = 2 * DQ // 128
NHD = DQ // 128
VW = DH + 1
OW = min(512, DM)
NOUT = DM // OW


def _declare_io(nc):
    t = {}
    t["xT"] = nc.dram_tensor("xT", [DM, S], F32R, kind="ExternalInput").ap()
    t["wqkT"] = nc.dram_tensor("wqkT", [DM, 2 * DQ], F32R, kind="ExternalInput").ap()
    t["wvT"] = nc.dram_tensor("wvT", [DM, DQ], F32R, kind="ExternalInput").ap()
    t["woT"] = nc.dram_tensor("woT", [DQ, DM], F32R, kind="ExternalInput").ap()
    t["bqk"] = nc.dram_tensor("bqk", [2 * DQ, 1], F32, kind="ExternalInput").ap()
    t["bv"] = nc.dram_tensor("bv", [128, DQ], F32, kind="ExternalInput").ap()
    t["out"] = nc.dram_tensor("out", [S, DM], F32, kind="ExternalOutput").ap()
    return t


def _build(ctx: ExitStack, tc: tile.TileContext, io: dict):
    nc = tc.nc

    const = ctx.enter_context(tc.tile_pool(name="const", bufs=1))
    work = ctx.enter_context(tc.tile_pool(name="work", bufs=1))
    psum = ctx.enter_context(tc.tile_pool(name="psum", bufs=1, space="PSUM"))

    # ---- constants / inputs ----
    xT = [const.tile([128, S], F32R, name=f"xT{c}") for c in range(NDM)]
    for c in range(NDM):
        nc.sync.dma_start(xT[c][:], io["xT"][c * 128 : (c + 1) * 128, :])

    wqk = [const.tile([128, 2 * DQ], F32R, name=f"wqk{c}") for c in range(NDM)]
    for c in range(NDM):
        nc.sync.dma_start(wqk[c][:], io["wqkT"][c * 128 : (c + 1) * 128, :])

    wv = [const.tile([128, DQ], F32R, name=f"wv{c}") for c in range(NDM)]
    for c in range(NDM):
        nc.sync.dma_start(wv[c][:], io["wvT"][c * 128 : (c + 1) * 128, :])

    wo = [const.tile([128, DM], F32R, name=f"wo{c}") for c in range(NHD)]
    for c in range(NHD):
        nc.sync.dma_start(wo[c][:], io["woT"][c * 128 : (c + 1) * 128, :])

    bqk = [const.tile([128, 1], F32, name=f"bqk{c}") for c in range(NQK)]
    for c in range(NQK):
        nc.sync.dma_start(bqk[c][:], io["bqk"][c * 128 : (c + 1) * 128, :])

    bv = const.tile([128, DQ], F32, name="bv")
    nc.sync.dma_start(bv[:], io["bv"][:])

    # triangular causal mask for the diagonal 128x128 sub-block:
    # tri[r, c] = (c - r >= 0)
    tri = const.tile([128, 128], F32R, name="tri")
    nc.gpsimd.memset(tri[:].bitcast(F32), 1.0)
    nc.gpsimd.affine_select(
        out=tri[:].bitcast(F32),
        in_=tri[:].bitcast(F32),
        compare_op=mybir.AluOpType.is_ge,
        fill=0.0,
        base=0,
        pattern=[[1, 128]],
        channel_multiplier=-1,
    )

    # ---- phase 1: q/k projection (transposed layout) ----
    qkT = [const.tile([128, S], F32R, name=f"qkT{b}") for b in range(NQK)]
    for ob in range(NQK):
        for sp in range(NSPAN):
            pqk = psum.tile([128, SPAN], F32, name=f"pqk_{ob}_{sp}", tag="po", bufs=4)
            for c in range(NDM):
                nc.tensor.matmul(
                    pqk[:],
                    wqk[c][:, ob * 128 : (ob + 1) * 128],
                    xT[c][:, sp * SPAN : (sp + 1) * SPAN],
                    start=(c == 0),
                    stop=(c == NDM - 1),
                )
            nc.vector.tensor_scalar_add(
                qkT[ob][:, sp * SPAN : (sp + 1) * SPAN], pqk[:], bqk[ob][:]
            )

    # ---- phase 2: v projection into v' (per-head + ones column) ----
    vp = [const.tile([128, HPC * VW], F32R, name=f"vp{sb}") for sb in range(NSB)]
    for sb in range(NSB):
        pv = psum.tile([128, DQ], F32, name=f"pv_{sb}", tag="po", bufs=4)
        for c in range(NDM):
            nc.tensor.matmul(
                pv[:],
                xT[c][:, sb * 128 : (sb + 1) * 128],
                wv[c][:],
                start=(c == 0),
                stop=(c == NDM - 1),
            )
        vdst = vp[sb][:, 0 : HPC * VW].rearrange("p (h w) -> p h w", w=VW)[:, :, 0:DH]
        nc.vector.tensor_add(
            vdst,
            pv[:].rearrange("p (h d) -> p h d", d=DH),
            bv[:].rearrange("p (h d) -> p h d", d=DH),
        )
        ones_cols = vp[sb][:, DH : HPC * VW : VW]
        nc.vector.memset(ones_cols.bitcast(F32), 1.0)

    # ---- phase 3+4: attention (flash, S^T layout) + interleaved out-proj ----
    # Per sk-block group: all HPC heads' score matmuls (uniform K=64 shape),
    # then all HPC heads' PV matmuls (uniform K=128 shape, distinct PSUM
    # banks), PVs lagging one group so the exp chain stays off PE's critical
    # path. Shape-uniform runs keep the PE array from draining between
    # matmuls (alternating K=64/K=128 measured 672 ns/mm vs 232 uniform).
    OT = [const.tile([128, S], F32R, name=f"OT{c}") for c in range(NHD)]
    for sp in range(NSPAN):
        den = work.tile([32 * (HPC - 1) + 1, SPAN], F32, name=f"den_{sp}", tag="den", bufs=1)
        nsb = (sp + 1) * SBS  # causal: sk blocks up to the span end
        pos = {}
        pts = {}
        oraw = {}

        def emit_scores(sb):
            for h in range(HPC):
                qt = qkT[h // 2]
                kt = qkT[NQK // 2 + h // 2]
                qrow = (h % 2) * 64
                ps = psum.tile(
                    [128, SPAN], F32, name=f"ps_{h}_{sp}_{sb}", tag="ps", bufs=4
                )
                nc.tensor.matmul(
                    ps[:],
                    kt[qrow : qrow + 64, sb * 128 : (sb + 1) * 128],
                    qt[qrow : qrow + 64, sp * SPAN : (sp + 1) * SPAN],
                    start=True,
                    stop=True,
                )
                pt = work.tile(
                    [128, SPAN], F32R, name=f"pt_{h}_{sp}_{sb}", tag="pt", bufs=8
                )
                pts[(h, sb)] = pt
                d = sb - sp * SBS
                if d < 0:
                    nc.scalar.activation(pt[:], ps[:], AF.Exp, scale=0.125)
                else:
                    # diagonal block: cols < 128*d fully masked, then one
                    # triangular 128x128 sub-block
                    if d > 0:
                        nc.vector.memset(pt[:, 0 : 128 * d].bitcast(F32), 0.0)
                    nc.scalar.activation(
                        pt[:, 128 * d : SPAN], ps[:, 128 * d : SPAN],
                        AF.Exp, scale=0.125,
                    )
                    nc.vector.tensor_mul(
                        pt[:, 128 * d : 128 * (d + 1)],
                        pt[:, 128 * d : 128 * (d + 1)],
                        tri[:],
                    )

        def emit_pvs(sb):
            for h in range(HPC):
                if sb == 0:
                    pos[h] = psum.tile(
                        [VW, SPAN], F32, name=f"po_{h}_{sp}", tag="po", bufs=4
                    )
                nc.tensor.matmul(
                    pos[h][:],
                    vp[sb][:, h * VW : (h + 1) * VW],
                    pts.pop((h, sb))[:],
                    start=(sb == 0),
                    stop=(sb == nsb - 1),
                )
                if sb == nsb - 1:
                    # copy (out^T | denom) to SBUF to free the PSUM bank early
                    orw = work.tile(
                        [VW, SPAN], F32, name=f"oraw_{h}_{sp}", tag="oraw", bufs=4
                    )
                    oraw[h] = orw
                    nc.vector.tensor_copy(orw[:], pos[h][:])
                    nc.vector.tensor_copy(den[32 * h : 32 * h + 1, :], orw[VW - 1 : VW, :])

        for i in range(nsb + 1):
            if i < nsb:
                emit_scores(i)
            if i >= 1:
                emit_pvs(i - 1)

        denr = work.tile([32 * (HPC - 1) + 1, SPAN], F32, name=f"denr_{sp}", tag="denr", bufs=1)
        # only rows 0/32/64/96 are meaningful; reciprocal of the garbage
        # rows in between is never read
        nc.vector.reciprocal(denr[:], den[:])
        for h in range(HPC):
            ot_tile = OT[(h * DH) // 128]
            orow = (h * DH) % 128
            rtmp = work.tile([1, SPAN], F32, name=f"rtmp_{h}_{sp}", tag="rtmp", bufs=2)
            # partition_broadcast needs a partition-0 source
            nc.vector.tensor_copy(rtmp[:], denr[32 * h : 32 * h + 1, :])
            recb = work.tile([DH, SPAN], F32, name=f"recb_{h}_{sp}", tag="recb", bufs=2)
            nc.gpsimd.partition_broadcast(recb[:], rtmp[0:1, :])
            nc.vector.tensor_mul(
                ot_tile[orow : orow + DH, sp * SPAN : (sp + 1) * SPAN],
                oraw[h][0:DH, :],
                recb[:],
            )
    # out projection for this span's sq blocks
        for qb in range(sp * SBS, (sp + 1) * SBS):
            ob = work.tile([128, DM], F32, name=f"ob_{qb}", tag="ob", bufs=2)
            for nh in range(NOUT):
                pot = psum.tile([128, OW], F32, name=f"pot_{qb}_{nh}", tag="po", bufs=4)
                for c in range(NHD):
                    nc.tensor.matmul(
                        pot[:],
                        OT[c][:, qb * 128 : (qb + 1) * 128],
                        wo[c][:, nh * OW : (nh + 1) * OW],
                        start=(c == 0),
                        stop=(c == NHD - 1),
                    )
                if (qb + nh) % 2 == 0:
                    nc.scalar.copy(ob[:, nh * OW : (nh + 1) * OW], pot[:])
                else:
                    nc.vector.tensor_copy(ob[:, nh * OW : (nh + 1) * OW], pot[:])
            nc.sync.dma_start(io["out"][qb * 128 : (qb + 1) * 128, :], ob[:])


_NC_CACHE = {}


def _get_compiled():
    if "nc" not in _NC_CACHE:
        nc = bacc.Bacc(
            "TRN2", target_bir_lowering=False, debug=False, num_devices=N_CORES
        )
        io = _declare_io(nc)
        with tile.TileContext(nc) as tc, ExitStack() as ctx:
            _build(ctx, tc, io)
        nc.compile()
        _NC_CACHE["nc"] = nc
    return _NC_CACHE["nc"]


def _prep_core_inputs(x, W_qkv, b_qkv, W_out, b_out, core_id):
    g = core_id // CPG
    lane = core_id % CPG
    h0 = lane * HPC
    r = slice(h0 * DH, (h0 + HPC) * DH)
    Wq = W_qkv[0 * DM : 1 * DM, :][r, :]
    Wk = W_qkv[1 * DM : 2 * DM, :][r, :]
    Wv = W_qkv[2 * DM : 3 * DM, :][r, :]
    bq = b_qkv[0 * DM + h0 * DH : 0 * DM + (h0 + HPC) * DH]
    bk = b_qkv[1 * DM + h0 * DH : 1 * DM + (h0 + HPC) * DH]
    bv_ = b_qkv[2 * DM + h0 * DH : 2 * DM + (h0 + HPC) * DH]
    return {
        "xT": np.ascontiguousarray(x[g].T.astype(np.float32)),
        "wqkT": np.ascontiguousarray(
            np.concatenate([Wq.T, Wk.T], axis=1).astype(np.float32)
        ),
        "wvT": np.ascontiguousarray(Wv.T.astype(np.float32)),
        "woT": np.ascontiguousarray(W_out[:, r].T.astype(np.float32)),
        "bqk": np.concatenate([bq, bk]).reshape(2 * DQ, 1).astype(np.float32),
        "bv": np.ascontiguousarray(
            np.broadcast_to(bv_.reshape(1, DQ), (128, DQ)).astype(np.float32)
        ),
    }


def kernel(x, W_qkv, b_qkv, W_out, b_out, _trace=False):
    x = np.asarray(x)
    W_qkv = np.asarray(W_qkv)
    b_qkv = np.asarray(b_qkv)
    W_out = np.asarray(W_out)
    b_out = np.asarray(b_out)

    nc = _get_compiled()
    in_maps = [
        _prep_core_inputs(x, W_qkv, b_qkv, W_out, b_out, c) for c in range(N_CORES)
    ]
    res = run_bass_kernel_spmd(nc, in_maps, list(range(N_CORES)), trace=_trace)

    out = np.empty((B, S, DM), dtype=np.float32)
    for g in range(B):
        acc = res.results[g * CPG]["out"].astype(np.float32)
        for lane in range(1, CPG):
            acc = acc + res.results[g * CPG + lane]["out"]
        out[g] = acc + b_out[None, :].astype(np.float32)

    if _trace:
        kernel.last_exec_time_ns = res.exec_time_ns
        kernel.last_results = res
    return out



# revision 13
# speedup vs baseline: 1.1429x; 1.1429x over previous
"""Multi-head causal self-attention (torch nn.MultiheadAttention semantics)
on 8 Trainium2 NeuronCores.

Problem: x [2, 2048, 1024], 16 heads, head dim 64, fp32, causal, p_drop=0.

Sharding: 2 batch groups x 4-way head tensor-parallel.
  core c: batch b = c // 4, heads [lane*4, lane*4+4) with lane = c % 4.
The host sums the 4 partial out-projections per batch and adds b_out.

v2 design (vs the phase-serial f32r baseline):
  - bf16 operands everywhere (PSUM accumulation stays fp32); rel-err gate is
    2e-2, measured ~1e-3. Halves DMA + SBUF, enables FWL weight loads.
  - One fully software-pipelined span loop (SPAN=256 of sq): q/k/v projection
    for the span, flash attention steps, softmax-normalize, out-projection,
    output DMA all emitted per span so Tile overlaps PE/ACT/DVE across
    phases and the PE never sees a >3us idle window (HAM stays at 8/8).
  - Scores for all 4 heads of a step land in ONE 4-bank PSUM tile (one bank
    per head); a single strided ACT Exp reads all 4 banks -> pt [128,1024]
    bf16. 72 activation calls total instead of 160.
  - K=64 score matmuls packed pairwise into array row-groups 0-63/64-127 via
    tile_position, so two heads' score matmuls run concurrently.
  - PV accumulators: 2 heads share one PSUM bank ([65, 2x256]); banks are
    pre-zeroed with DVE memset and all PV matmuls accumulate (start=False),
    which is scheduler-order-independent (no has_written bank clears).
  - Softmax denominator via the appended ones-column of v' (row 64 of po);
    reciprocal_approx_fast directly off PSUM, gpsimd partition-broadcast,
    DVE multiply writes normalized OT in bf16.
  - PSUM budget: 4 (scores) + 2 (PV) + 2 (projection ping-pong) = 8 banks.
"""

import os
from contextlib import ExitStack

import ml_dtypes
import numpy as np

import concourse.bass as bass
import concourse.tile as tile
from concourse import bacc, mybir
from concourse.bass_utils import run_bass_kernel_spmd

F32 = mybir.dt.float32
BF16 = mybir.dt.bfloat16
AF = mybir.ActivationFunctionType

B = 2
S = 2048
DM = 1024
N_HEADS = 16
DH = 64
N_CORES = 8
CPG = 4  # cores per group (tensor-parallel width over heads)
HPC = N_HEADS // CPG  # heads per core (4)
DQ = HPC * DH  # 256
SPAN = 256  # sq span per attention round
NSPAN = S // SPAN  # 8
SB = 128  # sk block
NSB = S // SB  # 16
NDM = DM // 128  # 8 dm row-tiles
NQK = 2 * DQ // 128  # 4 qkT tiles (q01, q23, k01, k23)
VW = DH + 1  # 65: per-head v width incl ones column
XCH = 512  # x load chunk (columns)


DEBUG = bool(os.environ.get("KDBG"))


def _declare_io(nc):
    t = {}
    if DEBUG:
        t["dbg_qkT"] = nc.dram_tensor("dbg_qkT", [2 * DQ, S], BF16, kind="ExternalOutput").ap()
        t["dbg_vp"] = nc.dram_tensor("dbg_vp", [128, HPC * VW], BF16, kind="ExternalOutput").ap()
        t["dbg_pt"] = nc.dram_tensor("dbg_pt", [128, HPC * SPAN], BF16, kind="ExternalOutput").ap()
        t["dbg_den"] = nc.dram_tensor("dbg_den", [2, 512], F32, kind="ExternalOutput").ap()
        t["dbg_po"] = nc.dram_tensor("dbg_po", [2 * VW, 512], F32, kind="ExternalOutput").ap()
        t["dbg_pt1"] = nc.dram_tensor("dbg_pt1", [128, HPC * SPAN], BF16, kind="ExternalOutput").ap()
        t["dbg_OT"] = nc.dram_tensor("dbg_OT", [DQ, S], BF16, kind="ExternalOutput").ap()
    t["xT"] = nc.dram_tensor("xT", [DM, S], BF16, kind="ExternalInput").ap()
    t["wqkT"] = nc.dram_tensor("wqkT", [DM, 2 * DQ], BF16, kind="ExternalInput").ap()
    t["wvT"] = nc.dram_tensor("wvT", [DM, DQ], BF16, kind="ExternalInput").ap()
    t["woT"] = nc.dram_tensor("woT", [DQ, DM], BF16, kind="ExternalInput").ap()
    t["bqk"] = nc.dram_tensor("bqk", [2 * DQ, 1], F32, kind="ExternalInput").ap()
    t["bv"] = nc.dram_tensor("bv", [128, DQ], BF16, kind="ExternalInput").ap()
    t["out"] = nc.dram_tensor("out", [S, DM], BF16, kind="ExternalOutput").ap()
    return t


def _build(ctx: ExitStack, tc: tile.TileContext, io: dict):
    nc = tc.nc

    const = ctx.enter_context(tc.tile_pool(name="const", bufs=1))
    work = ctx.enter_context(tc.tile_pool(name="work", bufs=1))
    psum = ctx.enter_context(tc.tile_pool(name="psum", bufs=1, space="PSUM"))

    # ---- input DMAs, in priority order ----
    wqk = [const.tile([128, 2 * DQ], BF16, name=f"wqk{c}") for c in range(NDM)]
    for c in range(NDM):
        nc.sync.dma_start(wqk[c][:], io["wqkT"][c * 128 : (c + 1) * 128, :])

    wv = [const.tile([128, DQ], BF16, name=f"wv{c}") for c in range(NDM)]
    for c in range(NDM):
        nc.sync.dma_start(wv[c][:], io["wvT"][c * 128 : (c + 1) * 128, :])

    bqk = [const.tile([128, 1], F32, name=f"bqk{c}") for c in range(NQK)]
    for c in range(NQK):
        nc.sync.dma_start(bqk[c][:], io["bqk"][c * 128 : (c + 1) * 128, :])

    bv = const.tile([128, DQ], BF16, name="bv")
    nc.sync.dma_start(bv[:], io["bv"][:])

    xT = [const.tile([128, S], BF16, name=f"xT{c}") for c in range(NDM)]
    for ch in range(S // XCH):
        for c in range(NDM):
            nc.sync.dma_start(
                xT[c][:, ch * XCH : (ch + 1) * XCH],
                io["xT"][c * 128 : (c + 1) * 128, ch * XCH : (ch + 1) * XCH],
            )
        if ch == 0:
            wo = [const.tile([128, DM], BF16, name=f"wo{c}") for c in range(DQ // 128)]
            for c in range(DQ // 128):
                nc.sync.dma_start(wo[c][:], io["woT"][c * 128 : (c + 1) * 128, :])

    # triangular causal mask for the diagonal 128x128 sub-block:
    # tri[r, c] = (c - r >= 0)
    tri = const.tile([128, 128], BF16, name="tri")
    nc.gpsimd.memset(tri[:], 1.0)
    nc.gpsimd.affine_select(
        out=tri[:],
        in_=tri[:],
        compare_op=mybir.AluOpType.is_ge,
        fill=0.0,
        base=0,
        pattern=[[1, 128]],
        channel_multiplier=-1,
    )

    # ---- persistent tiles ----
    # qkT tiles: 0=q heads(0,1), 1=q heads(2,3), 2=k heads(0,1), 3=k heads(2,3)
    qkT = [const.tile([128, S], BF16, name=f"qkT{b}") for b in range(NQK)]
    vp = [const.tile([128, HPC * VW], BF16, name=f"vp{sb}") for sb in range(NSB)]
    # OT tile c: rows 0:64 = head 2c, rows 64:128 = head 2c+1 (normalized out^T)
    OT = [const.tile([128, S], BF16, name=f"OT{c}") for c in range(HPC // 2)]

    # persistent PSUM: scores (4 banks; head h in cols [h*512, h*512+256)),
    # po (2 banks; pair p holds head 2p at cols 0:256, head 2p+1 at 256:512)
    ps4 = psum.tile([128, 2048], F32, name="ps4", tag="ps4", bufs=1)
    po = [
        psum.tile([VW, 512], F32, name=f"po{p}", tag=f"po{p}", bufs=1) for p in range(2)
    ]

    for sp in range(NSPAN):
        s0 = sp * SPAN
        sq = slice(s0, s0 + SPAN)

        # ---- q/k projection for this span ----
        for ob in range(NQK):
            pqk = psum.tile([128, SPAN], F32, name=f"pqk_{ob}_{sp}", tag="proj", bufs=2)
            for c in range(NDM):
                nc.tensor.matmul(
                    pqk[:],
                    wqk[c][:, ob * 128 : (ob + 1) * 128],
                    xT[c][:, sq],
                    start=(c == 0),
                    stop=(c == NDM - 1),
                )
            nc.vector.tensor_scalar_add(qkT[ob][:, sq], pqk[:], bqk[ob][:])

        # ---- v projection for this span's two sk blocks ----
        for sb in (2 * sp, 2 * sp + 1):
            pv = psum.tile([128, DQ], F32, name=f"pv_{sb}", tag="proj", bufs=2)
            for c in range(NDM):
                nc.tensor.matmul(
                    pv[:],
                    xT[c][:, sb * 128 : (sb + 1) * 128],
                    wv[c][:],
                    start=(c == 0),
                    stop=(c == NDM - 1),
                )
            vdst = vp[sb][:, 0 : HPC * VW].rearrange("p (h w) -> p h w", w=VW)[
                :, :, 0:DH
            ]
            nc.vector.tensor_add(
                vdst,
                pv[:].rearrange("p (h d) -> p h d", d=DH),
                bv[:].rearrange("p (h d) -> p h d", d=DH),
            )
            ones_cols = vp[sb][:, DH : HPC * VW : VW]
            nc.vector.memset(ones_cols, 1.0)

        # ---- attention over sk blocks 0..2sp+1 ----
        nsb = 2 * (sp + 1)
        # pre-zero PV accumulator banks; all PV matmuls accumulate onto 0
        # (order-independent; no has_written bank-wide clears)
        for p in range(2):
            nc.vector.memset(po[p][:], 0.0)

        pts = {}

        def emit_scores(sb, sp=sp, s0=s0):
            d = sb - 2 * sp
            off = 128 if d == 1 else 0  # cols 0:128 fully masked on d==1
            for pr in range(2):
                qt = qkT[pr]
                kt = qkT[2 + pr]
                for sub in range(2):  # head 2*pr+sub, array row-group sub
                    h = 2 * pr + sub
                    r0, r1 = sub * 64, sub * 64 + 64
                    nc.tensor.matmul(
                        ps4[:, h * 512 + off : h * 512 + SPAN],
                        kt[r0:r1, sb * 128 : (sb + 1) * 128],
                        qt[r0:r1, s0 + off : s0 + SPAN],
                        start=True,
                        stop=True,
                        tile_position=(sub * 64, 0),
                    )
            pt = work.tile([128, HPC * SPAN], BF16, name=f"pt_{sp}_{sb}", tag="pt", bufs=3)
            pts[sb] = pt
            nc.scalar.activation(
                pt[:].rearrange("p (h w) -> p h w", w=SPAN),
                ps4[:].rearrange("p (h w) -> p h w", w=512)[:, :, 0:SPAN],
                AF.Exp,
                scale=0.125,
            )
            if d >= 0:
                if d == 1:
                    # cols 0:128 of every head's slice are fully masked
                    nc.vector.memset(
                        pt[:].rearrange("p (h w) -> p h w", w=SPAN)[:, :, 0:128], 0.0
                    )
                # triangular sub-block at cols [128d, 128d+128)
                for h in range(HPC):
                    o = h * SPAN + 128 * d
                    nc.vector.tensor_mul(pt[:, o : o + 128], pt[:, o : o + 128], tri[:])
            if DEBUG and sp == 0 and sb == 0:
                nc.sync.dma_start(io["dbg_pt"][:], pt[:])
            if DEBUG and sp == 0 and sb == 1:
                nc.sync.dma_start(io["dbg_pt1"][:], pt[:])

        def emit_pvs(sb, nsb=nsb):
            pt = pts.pop(sb)
            for h in range(HPC):
                nc.tensor.matmul(
                    po[h // 2][:, (h % 2) * SPAN : (h % 2 + 1) * SPAN],
                    vp[sb][:, h * VW : (h + 1) * VW],
                    pt[:, h * SPAN : (h + 1) * SPAN],
                    start=False,
                    stop=(sb == nsb - 1),
                    skip_group_check=True,
                )

        for i in range(nsb + 1):
            if i < nsb:
                emit_scores(i)
            if i >= 1:
                emit_pvs(i - 1)

        # ---- normalize: OT[h-rows, span] = po_v / po_denom ----
        for p in range(2):
            # custom-DVE ops mishandle nonzero base partitions: stage the
            # denominator row (partition 64) to a partition-0 tile first
            den_c = work.tile([1, 512], F32, name=f"den_{p}_{sp}", tag="den", bufs=2)
            nc.vector.tensor_copy(den_c[:], po[p][VW - 1 : VW, :])
            denr = work.tile([1, 512], F32, name=f"denr_{p}_{sp}", tag="denr", bufs=2)
            nc.vector.reciprocal_approx_fast(denr[:], den_c[:])
            if DEBUG and sp == 0:
                nc.sync.dma_start(io["dbg_den"][p : p + 1, :], denr[:])
                po_sb = work.tile([VW, 512], F32, name=f"po_sb_{p}", tag=f"po_sb{p}", bufs=1)
                nc.vector.tensor_copy(po_sb[:], po[p][:])
                nc.sync.dma_start(io["dbg_po"][p * VW : (p + 1) * VW, :], po_sb[:])
            for sub in range(2):
                h = 2 * p + sub
                recb = work.tile([DH, SPAN], F32, name=f"recb_{h}_{sp}", tag="recb", bufs=4)
                nc.gpsimd.partition_broadcast(
                    recb[:], denr[0:1, sub * SPAN : (sub + 1) * SPAN]
                )
                nc.vector.tensor_mul(
                    OT[p][sub * 64 : sub * 64 + 64, sq],
                    po[p][0:DH, sub * SPAN : (sub + 1) * SPAN],
                    recb[:],
                )

        # ---- out projection for this span's two row blocks ----
        for qb in (2 * sp, 2 * sp + 1):
            ob_t = work.tile([128, DM], BF16, name=f"ob_{qb}", tag="ob", bufs=2)
            for nh in range(2):
                pot = psum.tile([128, 512], F32, name=f"pot_{qb}_{nh}", tag="proj", bufs=2)
                for c in range(HPC // 2):
                    nc.tensor.matmul(
                        pot[:],
                        OT[c][:, qb * 128 : (qb + 1) * 128],
                        wo[c][:, nh * 512 : (nh + 1) * 512],
                        start=(c == 0),
                        stop=(c == HPC // 2 - 1),
                    )
                nc.any.tensor_copy(ob_t[:, nh * 512 : (nh + 1) * 512], pot[:])
            nc.sync.dma_start(io["out"][qb * 128 : (qb + 1) * 128, :], ob_t[:])

    if DEBUG:
        for b in range(NQK):
            nc.sync.dma_start(io["dbg_qkT"][b * 128 : (b + 1) * 128, :], qkT[b][:])
        nc.sync.dma_start(io["dbg_vp"][:], vp[0][:])
        for c in range(HPC // 2):
            nc.sync.dma_start(io["dbg_OT"][c * 128 : (c + 1) * 128, :], OT[c][:])


_NC_CACHE = {}


def _get_compiled():
    if "nc" not in _NC_CACHE:
        nc = bacc.Bacc(
            "TRN2", target_bir_lowering=False, debug=False, num_devices=N_CORES
        )
        io = _declare_io(nc)
        with tile.TileContext(nc) as tc, ExitStack() as ctx:
            _build(ctx, tc, io)
        nc.compile()
        _NC_CACHE["nc"] = nc
    return _NC_CACHE["nc"]


def _bf16(a):
    return np.ascontiguousarray(a.astype(ml_dtypes.bfloat16))


def _prep_core_inputs(x, W_qkv, b_qkv, W_out, b_out, core_id):
    g = core_id // CPG
    lane = core_id % CPG
    h0 = lane * HPC
    r = slice(h0 * DH, (h0 + HPC) * DH)
    Wq = W_qkv[0 * DM : 1 * DM, :][r, :]
    Wk = W_qkv[1 * DM : 2 * DM, :][r, :]
    Wv = W_qkv[2 * DM : 3 * DM, :][r, :]
    bq = b_qkv[0 * DM + h0 * DH : 0 * DM + (h0 + HPC) * DH]
    bk = b_qkv[1 * DM + h0 * DH : 1 * DM + (h0 + HPC) * DH]
    bv_ = b_qkv[2 * DM + h0 * DH : 2 * DM + (h0 + HPC) * DH]
    return {
        "xT": _bf16(x[g].T),
        "wqkT": _bf16(np.concatenate([Wq.T, Wk.T], axis=1)),
        "wvT": _bf16(Wv.T),
        "woT": _bf16(W_out[:, r].T),
        "bqk": np.concatenate([bq, bk]).reshape(2 * DQ, 1).astype(np.float32),
        "bv": _bf16(np.broadcast_to(bv_.reshape(1, DQ), (128, DQ))),
    }


def kernel(x, W_qkv, b_qkv, W_out, b_out, _trace=False):
    x = np.asarray(x)
    W_qkv = np.asarray(W_qkv)
    b_qkv = np.asarray(b_qkv)
    W_out = np.asarray(W_out)
    b_out = np.asarray(b_out)

    nc = _get_compiled()
    in_maps = [
        _prep_core_inputs(x, W_qkv, b_qkv, W_out, b_out, c) for c in range(N_CORES)
    ]
    res = run_bass_kernel_spmd(nc, in_maps, list(range(N_CORES)), trace=_trace)

    out = np.empty((B, S, DM), dtype=np.float32)
    for g in range(B):
        acc = res.results[g * CPG]["out"].astype(np.float32)
        for lane in range(1, CPG):
            acc = acc + res.results[g * CPG + lane]["out"].astype(np.float32)
        out[g] = acc + b_out[None, :].astype(np.float32)

    globals()["kernel_last_res"] = res
    if _trace:
        kernel.last_exec_time_ns = res.exec_time_ns
        kernel.last_results = res
    return out


# revision 20
# speedup vs baseline: 1.5191x; 1.3292x over previous
"""Multi-head causal self-attention (torch nn.MultiheadAttention semantics)
on 8 Trainium2 NeuronCores.

Problem: x [2, 2048, 1024], 16 heads, head dim 64, fp32, causal, p_drop=0.

Sharding: 2 batch groups x 4-way head tensor-parallel.
  core c: batch b = c // 4, heads [lane*4, lane*4+4) with lane = c % 4.
The host sums the 4 partial out-projections per batch and adds b_out.

v2 design (vs the phase-serial f32r baseline):
  - bf16 operands everywhere (PSUM accumulation stays fp32); rel-err gate is
    2e-2, measured ~1e-3. Halves DMA + SBUF, enables FWL weight loads.
  - One fully software-pipelined span loop (SPAN=256 of sq): q/k/v projection
    for the span, flash attention steps, softmax-normalize, out-projection,
    output DMA all emitted per span so Tile overlaps PE/ACT/DVE across
    phases and the PE never sees a >3us idle window (HAM stays at 8/8).
  - Scores for all 4 heads of a step land in ONE 4-bank PSUM tile (one bank
    per head); a single strided ACT Exp reads all 4 banks -> pt [128,1024]
    bf16. 72 activation calls total instead of 160.
  - K=64 score matmuls packed pairwise into array row-groups 0-63/64-127 via
    tile_position, so two heads' score matmuls run concurrently.
  - PV accumulators: 2 heads share one PSUM bank ([65, 2x256]); banks are
    pre-zeroed with DVE memset and all PV matmuls accumulate (start=False),
    which is scheduler-order-independent (no has_written bank clears).
  - Softmax denominator via the appended ones-column of v' (row 64 of po);
    reciprocal_approx_fast directly off PSUM, gpsimd partition-broadcast,
    DVE multiply writes normalized OT in bf16.
  - PSUM budget: 4 (scores) + 2 (PV) + 2 (projection ping-pong) = 8 banks.
"""

import os
from contextlib import ExitStack

import ml_dtypes
import numpy as np

import concourse.bass as bass
import concourse.tile as tile
from concourse import bacc, mybir
from concourse.bass_utils import run_bass_kernel_spmd

F32 = mybir.dt.float32
BF16 = mybir.dt.bfloat16
AF = mybir.ActivationFunctionType

B = 2
S = 2048
DM = 1024
N_HEADS = 16
DH = 64
N_CORES = 8
CPG = 4  # cores per group (tensor-parallel width over heads)
HPC = N_HEADS // CPG  # heads per core (4)
DQ = HPC * DH  # 256
SPAN = 256  # sq span per attention round
NSPAN = S // SPAN  # 8
SB = 128  # sk block
NSB = S // SB  # 16
NDM = DM // 128  # 8 dm row-tiles
NQK = 2 * DQ // 128  # 4 qkT tiles (q01, q23, k01, k23)
VW = DH + 1  # 65: per-head v width incl ones column
XCH = 512  # x load chunk (columns)


DEBUG = bool(os.environ.get("KDBG"))


def _declare_io(nc):
    t = {}
    # consolidated host-side layouts: one DMA per tensor (4 for x), issue
    # overhead on the sync queue is ~600ns per dma_start
    if DEBUG:
        t["dbg_qkT"] = nc.dram_tensor("dbg_qkT", [2 * DQ, S], BF16, kind="ExternalOutput").ap()
        t["dbg_vp"] = nc.dram_tensor("dbg_vp", [128, HPC * VW], BF16, kind="ExternalOutput").ap()
        t["dbg_pt"] = nc.dram_tensor("dbg_pt", [128, HPC * SPAN], BF16, kind="ExternalOutput").ap()
        t["dbg_den"] = nc.dram_tensor("dbg_den", [2, 512], F32, kind="ExternalOutput").ap()
        t["dbg_po"] = nc.dram_tensor("dbg_po", [2 * VW, 512], F32, kind="ExternalOutput").ap()
        t["dbg_pt1"] = nc.dram_tensor("dbg_pt1", [128, HPC * SPAN], BF16, kind="ExternalOutput").ap()
        t["dbg_OT"] = nc.dram_tensor("dbg_OT", [DQ, S], BF16, kind="ExternalOutput").ap()
    # x as [128, NDM, S]: partition p, dm-tile c, seq s  (from xT[c*128+p, s])
    t["xT"] = nc.dram_tensor("xT", [128, NDM, S], BF16, kind="ExternalInput").ap()
    # wqk as [128, NDM, 2*DQ], wv as [128, NDM, DQ] (same per-tile packing)
    t["wqkT"] = nc.dram_tensor("wqkT", [128, NDM * 2 * DQ], BF16, kind="ExternalInput").ap()
    t["wvT"] = nc.dram_tensor("wvT", [128, NDM * DQ], BF16, kind="ExternalInput").ap()
    t["woT"] = nc.dram_tensor("woT", [128, 2 * DM], BF16, kind="ExternalInput").ap()
    t["bqk"] = nc.dram_tensor("bqk", [128, NQK], F32, kind="ExternalInput").ap()
    t["bv"] = nc.dram_tensor("bv", [128, DQ], BF16, kind="ExternalInput").ap()
    t["out"] = nc.dram_tensor("out", [S, DM], BF16, kind="ExternalOutput").ap()
    return t


def _build(ctx: ExitStack, tc: tile.TileContext, io: dict):
    nc = tc.nc

    const = ctx.enter_context(tc.tile_pool(name="const", bufs=1))
    work = ctx.enter_context(tc.tile_pool(name="work", bufs=1))
    psum = ctx.enter_context(tc.tile_pool(name="psum", bufs=1, space="PSUM"))

    # ---- input DMAs, in priority order (one dma_start per tensor/chunk) ----
    wqk_all = const.tile([128, NDM * 2 * DQ], BF16, name="wqk_all")
    nc.sync.dma_start(wqk_all[:], io["wqkT"][:])
    wqk = [wqk_all[:, c * 2 * DQ : (c + 1) * 2 * DQ] for c in range(NDM)]

    wv_all = const.tile([128, NDM * DQ], BF16, name="wv_all")
    nc.sync.dma_start(wv_all[:], io["wvT"][:])
    wv = [wv_all[:, c * DQ : (c + 1) * DQ] for c in range(NDM)]

    bqk_all = const.tile([128, NQK], F32, name="bqk_all")
    nc.sync.dma_start(bqk_all[:], io["bqk"][:])
    bqk = [bqk_all[:, c : c + 1] for c in range(NQK)]

    bv = const.tile([128, DQ], BF16, name="bv")
    nc.sync.dma_start(bv[:], io["bv"][:])

    xT_all = const.tile([128, NDM * S], BF16, name="xT_all")
    xT = [xT_all[:, c * S : (c + 1) * S] for c in range(NDM)]
    xT3 = xT_all[:].rearrange("p (c s) -> p c s", s=S)
    io_x3 = io["xT"]
    for ch in range(S // XCH):
        nc.sync.dma_start(
            xT3[:, :, ch * XCH : (ch + 1) * XCH],
            io_x3[:, :, ch * XCH : (ch + 1) * XCH],
        )
        if ch == 0:
            wo_all = const.tile([128, 2 * DM], BF16, name="wo_all")
            nc.sync.dma_start(wo_all[:], io["woT"][:])
            wo = [wo_all[:, c * DM : (c + 1) * DM] for c in range(DQ // 128)]

    # triangular causal mask for the diagonal 128x128 sub-block:
    # tri[r, c] = (c - r >= 0)
    tri = const.tile([128, 128], BF16, name="tri")
    nc.gpsimd.memset(tri[:], 1.0)
    nc.gpsimd.affine_select(
        out=tri[:],
        in_=tri[:],
        compare_op=mybir.AluOpType.is_ge,
        fill=0.0,
        base=0,
        pattern=[[1, 128]],
        channel_multiplier=-1,
    )

    # ---- persistent tiles ----
    # qkT tiles: 0=q heads(0,1), 1=q heads(2,3), 2=k heads(0,1), 3=k heads(2,3)
    qkT = [const.tile([128, S], BF16, name=f"qkT{b}") for b in range(NQK)]
    vp = [const.tile([128, HPC * VW], BF16, name=f"vp{sb}") for sb in range(NSB)]
    # OT tile c: rows 0:64 = head 2c, rows 64:128 = head 2c+1 (normalized out^T)
    OT = [const.tile([128, S], BF16, name=f"OT{c}") for c in range(HPC // 2)]

    # persistent PSUM: scores (4 banks; head h in cols [h*512, h*512+256)),
    # po (2 banks; pair p holds head 2p at cols 0:256, head 2p+1 at 256:512)
    ps4 = psum.tile([128, 2048], F32, name="ps4", tag="ps4", bufs=1)
    po = [
        psum.tile([VW, 512], F32, name=f"po{p}", tag=f"po{p}", bufs=1) for p in range(2)
    ]

    def emit_outproj(sp):
        # out projection for span sp's two row blocks (emitted one span late
        # so its PSUM-slot allocation never chains the next span's q/k
        # projection behind this span's attention)
        for qb in (2 * sp, 2 * sp + 1):
            ob_t = work.tile([128, DM], BF16, name=f"ob_{qb}", tag="ob", bufs=2)
            for nh in range(2):
                pot = psum.tile([128, 512], F32, name=f"pot_{qb}_{nh}", tag="proj", bufs=2)
                for c in range(HPC // 2):
                    nc.tensor.matmul(
                        pot[:],
                        OT[c][:, qb * 128 : (qb + 1) * 128],
                        wo[c][:, nh * 512 : (nh + 1) * 512],
                        start=(c == 0),
                        stop=(c == HPC // 2 - 1),
                    )
                nc.vector.tensor_copy(ob_t[:, nh * 512 : (nh + 1) * 512], pot[:])
            nc.sync.dma_start(io["out"][qb * 128 : (qb + 1) * 128, :], ob_t[:])

    for sp in range(NSPAN):
        s0 = sp * SPAN
        sq = slice(s0, s0 + SPAN)

        # ---- q/k projection for this span ----
        for ob in range(NQK):
            pqk = psum.tile([128, SPAN], F32, name=f"pqk_{ob}_{sp}", tag="proj", bufs=2)
            for c in range(NDM):
                nc.tensor.matmul(
                    pqk[:],
                    wqk[c][:, ob * 128 : (ob + 1) * 128],
                    xT[c][:, sq],
                    start=(c == 0),
                    stop=(c == NDM - 1),
                )
            nc.vector.tensor_scalar_add(qkT[ob][:, sq], pqk[:], bqk[ob][:])

        # ---- v projection for this span's two sk blocks ----
        for sb in (2 * sp, 2 * sp + 1):
            pv = psum.tile([128, DQ], F32, name=f"pv_{sb}", tag="proj", bufs=2)
            for c in range(NDM):
                nc.tensor.matmul(
                    pv[:],
                    xT[c][:, sb * 128 : (sb + 1) * 128],
                    wv[c][:],
                    start=(c == 0),
                    stop=(c == NDM - 1),
                )
            vdst = vp[sb][:, 0 : HPC * VW].rearrange("p (h w) -> p h w", w=VW)[
                :, :, 0:DH
            ]
            nc.vector.tensor_add(
                vdst,
                pv[:].rearrange("p (h d) -> p h d", d=DH),
                bv[:].rearrange("p (h d) -> p h d", d=DH),
            )
            ones_cols = vp[sb][:, DH : HPC * VW : VW]
            nc.vector.memset(ones_cols, 1.0)

        if sp > 0:
            emit_outproj(sp - 1)

        # ---- attention over sk blocks 0..2sp+1 ----
        nsb = 2 * (sp + 1)
        # pre-zero PV accumulator banks; all PV matmuls accumulate onto 0
        # (order-independent; no has_written bank-wide clears)
        for p in range(2):
            nc.vector.memset(po[p][:], 0.0)

        pts = {}

        def emit_scores(sb, sp=sp, s0=s0):
            d = sb - 2 * sp
            off = 128 if d == 1 else 0  # cols 0:128 fully masked on d==1
            for pr in range(2):
                qt = qkT[pr]
                kt = qkT[2 + pr]
                for sub in range(2):  # head 2*pr+sub, array row-group sub
                    h = 2 * pr + sub
                    r0, r1 = sub * 64, sub * 64 + 64
                    nc.tensor.matmul(
                        ps4[:, h * 512 + off : h * 512 + SPAN],
                        kt[r0:r1, sb * 128 : (sb + 1) * 128],
                        qt[r0:r1, s0 + off : s0 + SPAN],
                        start=True,
                        stop=True,
                        tile_position=(sub * 64, 0),
                    )
            pt = work.tile([128, HPC * SPAN], BF16, name=f"pt_{sp}_{sb}", tag="pt", bufs=3)
            pts[sb] = pt
            nc.scalar.activation(
                pt[:].rearrange("p (h w) -> p h w", w=SPAN),
                ps4[:].rearrange("p (h w) -> p h w", w=512)[:, :, 0:SPAN],
                AF.Exp,
                scale=0.125,
            )
            if d >= 0:
                if d == 1:
                    # cols 0:128 of every head's slice are fully masked
                    nc.vector.memset(
                        pt[:].rearrange("p (h w) -> p h w", w=SPAN)[:, :, 0:128], 0.0
                    )
                # triangular sub-block at cols [128d, 128d+128)
                for h in range(HPC):
                    o = h * SPAN + 128 * d
                    nc.vector.tensor_mul(pt[:, o : o + 128], pt[:, o : o + 128], tri[:])
            if DEBUG and sp == 0 and sb == 0:
                nc.sync.dma_start(io["dbg_pt"][:], pt[:])
            if DEBUG and sp == 0 and sb == 1:
                nc.sync.dma_start(io["dbg_pt1"][:], pt[:])

        def emit_pvs(sb, nsb=nsb):
            pt = pts.pop(sb)
            for h in range(HPC):
                nc.tensor.matmul(
                    po[h // 2][:, (h % 2) * SPAN : (h % 2 + 1) * SPAN],
                    vp[sb][:, h * VW : (h + 1) * VW],
                    pt[:, h * SPAN : (h + 1) * SPAN],
                    start=False,
                    stop=(sb == nsb - 1),
                    skip_group_check=True,
                )

        for i in range(nsb + 1):
            if i < nsb:
                emit_scores(i)
            if i >= 1:
                emit_pvs(i - 1)

        # ---- normalize: OT[h-rows, span] = po_v / po_denom ----
        for p in range(2):
            # custom-DVE ops mishandle nonzero base partitions: stage the
            # denominator row (partition 64) to a partition-0 tile first
            den_c = work.tile([1, 512], F32, name=f"den_{p}_{sp}", tag="den", bufs=2)
            nc.vector.tensor_copy(den_c[:], po[p][VW - 1 : VW, :])
            denr = work.tile([1, 512], F32, name=f"denr_{p}_{sp}", tag="denr", bufs=2)
            nc.vector.reciprocal_approx_fast(denr[:], den_c[:])
            if DEBUG and sp == 0:
                nc.sync.dma_start(io["dbg_den"][p : p + 1, :], denr[:])
                po_sb = work.tile([VW, 512], F32, name=f"po_sb_{p}", tag=f"po_sb{p}", bufs=1)
                nc.vector.tensor_copy(po_sb[:], po[p][:])
                nc.sync.dma_start(io["dbg_po"][p * VW : (p + 1) * VW, :], po_sb[:])
            for sub in range(2):
                h = 2 * p + sub
                recb = work.tile([DH, SPAN], F32, name=f"recb_{h}_{sp}", tag="recb", bufs=4)
                nc.gpsimd.partition_broadcast(
                    recb[:], denr[0:1, sub * SPAN : (sub + 1) * SPAN]
                )
                nc.vector.tensor_mul(
                    OT[p][sub * 64 : sub * 64 + 64, sq],
                    po[p][0:DH, sub * SPAN : (sub + 1) * SPAN],
                    recb[:],
                )

    emit_outproj(NSPAN - 1)

    if DEBUG:
        for b in range(NQK):
            nc.sync.dma_start(io["dbg_qkT"][b * 128 : (b + 1) * 128, :], qkT[b][:])
        nc.sync.dma_start(io["dbg_vp"][:], vp[0][:])
        for c in range(HPC // 2):
            nc.sync.dma_start(io["dbg_OT"][c * 128 : (c + 1) * 128, :], OT[c][:])


_NC_CACHE = {}


def _get_compiled():
    if "nc" not in _NC_CACHE:
        nc = bacc.Bacc(
            "TRN2", target_bir_lowering=False, debug=False, num_devices=N_CORES
        )
        io = _declare_io(nc)
        with tile.TileContext(nc) as tc, ExitStack() as ctx:
            _build(ctx, tc, io)
        nc.compile()
        _NC_CACHE["nc"] = nc
    return _NC_CACHE["nc"]


def _bf16(a):
    return np.ascontiguousarray(a.astype(ml_dtypes.bfloat16))


def _prep_core_inputs(x, W_qkv, b_qkv, W_out, b_out, core_id):
    g = core_id // CPG
    lane = core_id % CPG
    h0 = lane * HPC
    r = slice(h0 * DH, (h0 + HPC) * DH)
    Wq = W_qkv[0 * DM : 1 * DM, :][r, :]
    Wk = W_qkv[1 * DM : 2 * DM, :][r, :]
    Wv = W_qkv[2 * DM : 3 * DM, :][r, :]
    bq = b_qkv[0 * DM + h0 * DH : 0 * DM + (h0 + HPC) * DH]
    bk = b_qkv[1 * DM + h0 * DH : 1 * DM + (h0 + HPC) * DH]
    bv_ = b_qkv[2 * DM + h0 * DH : 2 * DM + (h0 + HPC) * DH]
    def tilepack(a):  # [R, C] with R = n*128  ->  [128, n, C]
        n = a.shape[0] // 128
        return a.reshape(n, 128, a.shape[1]).transpose(1, 0, 2)

    return {
        "xT": _bf16(tilepack(x[g].T)),
        "wqkT": _bf16(tilepack(np.concatenate([Wq.T, Wk.T], axis=1)).reshape(128, -1)),
        "wvT": _bf16(tilepack(Wv.T).reshape(128, -1)),
        "woT": _bf16(tilepack(W_out[:, r].T).reshape(128, -1)),
        "bqk": np.ascontiguousarray(
            np.concatenate([bq, bk]).reshape(NQK, 128).T.astype(np.float32)
        ),
        "bv": _bf16(np.broadcast_to(bv_.reshape(1, DQ), (128, DQ))),
    }


def kernel(x, W_qkv, b_qkv, W_out, b_out, _trace=False):
    x = np.asarray(x)
    W_qkv = np.asarray(W_qkv)
    b_qkv = np.asarray(b_qkv)
    W_out = np.asarray(W_out)
    b_out = np.asarray(b_out)

    nc = _get_compiled()
    in_maps = [
        _prep_core_inputs(x, W_qkv, b_qkv, W_out, b_out, c) for c in range(N_CORES)
    ]
    res = run_bass_kernel_spmd(nc, in_maps, list(range(N_CORES)), trace=_trace)

    out = np.empty((B, S, DM), dtype=np.float32)
    for g in range(B):
        acc = res.results[g * CPG]["out"].astype(np.float32)
        for lane in range(1, CPG):
            acc = acc + res.results[g * CPG + lane]["out"].astype(np.float32)
        out[g] = acc + b_out[None, :].astype(np.float32)

    globals()["kernel_last_res"] = res
    if _trace:
        kernel.last_exec_time_ns = res.exec_time_ns
        kernel.last_results = res
    return out
